# revision 1
# baseline (speedup 1.0000x reference)
"""Kuramoto layer Bass/Tile kernel for 8 Trainium2 NeuronCores.

Math: coupling[b,i,d] = (1/N) * sum_j W[b,i,j] * sin(theta[b,j,d] - theta[b,i,d] - alpha[b,i,j])
Using sin(tj - ti - a) = cos(ti)*(sin(tj)cos(a) - cos(tj)sin(a)) - sin(ti)*(cos(tj)cos(a) + sin(tj)sin(a)):
  A[i,d] = sum_j (W cos a)[i,j] S[j,d] - (W sin a)[i,j] C[j,d]
  B[i,d] = sum_j (W cos a)[i,j] C[j,d] + (W sin a)[i,j] S[j,d]
  coupling = cos(ti) * A - sin(ti) * B
  out = normalize(gamma + coupling/N, dim=-1, eps=1e-6)

sin/cos of alpha via half-angle identities (valid since |alpha| < 2*pi here, so
the ACT Sin args alpha/2, alpha/4 stay inside the table's [-pi, pi] domain):
  cos a = 1 - 2*sin^2(a/2)
  sin a = sin(a/2) * (2 - 4*sin^2(a/4))

Sharding: core c handles batch c//4, i-rows (c%4)*1024 .. +1024. theta (j-side)
is replicated per batch. No cross-core communication.

Per-core pipeline (~balanced at the 32 MiB/core HBM roofline):
  ACT : Wb = bf16(W)                            (natural layout)
  PE  : 128x128 transposes of alpha (fp32) and Wb (bf16) into PSUM
  ACT : q2 = Sin(0.5*alphaT); q4 = Sin(0.25*alphaT)   (PSUM -> bf16 SBUF)
  DVE : cc = 1-2*q2^2; ss = q2*(2-4*q4^2); UT = WbT*cc; VT = WbT*ss  (bf16 2x)
  PE  : psum[8,512] += [S|C]^T @ UT + [-C|S]^T @ VT over all j (bf16 matmuls)
  finish: transpose [8,512] back to [128,8], combine with cos/sin(theta_i),
  add gamma, normalize, DMA out.
"""

import sys

if "/opt/trn_rl_repo" not in sys.path:
    sys.path.insert(0, "/opt/trn_rl_repo")

import math

import numpy as np

B, N, D = 2, 4096, 4
N_CORES = 8
CORES_PER_BATCH = N_CORES // B          # 4
ROWS = B * N // N_CORES                 # 1024 i-rows per core
P = 128
SLAB = 512                              # i-slab (matmul moving width)
NSLAB = ROWS // SLAB                    # 2
JC = 1024                               # j chunk
NCHUNK = N // JC                        # 4
NB = ROWS // P                          # 8 row-blocks per core
PI = math.pi
EPS = 1e-6
GLOBAL_COUPLING = 1.0
STEP_SIZE = 1.0
GAMMA_STRENGTH = 1.0

_CACHE = {}


def _build():
    from concourse import bacc, mybir
    import concourse.tile as tile
    from concourse.masks import make_identity

    f32 = mybir.dt.float32
    bf16 = mybir.dt.bfloat16
    Alu = mybir.AluOpType
    Act = mybir.ActivationFunctionType

    nc = bacc.Bacc("TRN2", target_bir_lowering=False, debug=False,
                   num_devices=N_CORES)

    w_d = nc.dram_tensor("w", [ROWS, N], f32, kind="ExternalInput")
    a_d = nc.dram_tensor("alpha", [ROWS, N], f32, kind="ExternalInput")
    th_d = nc.dram_tensor("theta", [N, D], f32, kind="ExternalInput")
    thi_d = nc.dram_tensor("theta_i", [ROWS, D], f32, kind="ExternalInput")
    gm_d = nc.dram_tensor("gamma", [ROWS, D], f32, kind="ExternalInput")
    out_d = nc.dram_tensor("out", [ROWS, D], f32, kind="ExternalOutput")

    JT = N // P  # 32 j-tiles

    def sincos(pool, src, width, tag):
        """f32 sin/cos of src [P, width] via half-angle; returns (sin, cos)."""
        q2 = pool.tile([P, width], f32, tag=f"{tag}q2")
        q4 = pool.tile([P, width], f32, tag=f"{tag}q4")
        nc.scalar.activation(q2[:], src, Act.Sin, scale=0.5)
        nc.scalar.activation(q4[:], src, Act.Sin, scale=0.25)
        cos_t = pool.tile([P, width], f32, tag=f"{tag}cos")
        r2 = pool.tile([P, width], f32, tag=f"{tag}r2")
        nc.vector.tensor_tensor(out=r2[:], in0=q2[:], in1=q2[:], op=Alu.mult)
        nc.vector.tensor_scalar(cos_t[:], r2[:], -2.0, 1.0, Alu.mult, Alu.add)
        r4 = pool.tile([P, width], f32, tag=f"{tag}r4")
        ch = pool.tile([P, width], f32, tag=f"{tag}ch")
        nc.vector.tensor_tensor(out=r4[:], in0=q4[:], in1=q4[:], op=Alu.mult)
        nc.vector.tensor_scalar(ch[:], r4[:], -4.0, 2.0, Alu.mult, Alu.add)
        sin_t = pool.tile([P, width], f32, tag=f"{tag}sin")
        nc.vector.tensor_tensor(out=sin_t[:], in0=q2[:], in1=ch[:], op=Alu.mult)
        return sin_t, cos_t

    with tile.TileContext(nc) as tc:
        with tc.tile_pool(name="const", bufs=1) as cpool, \
             tc.tile_pool(name="wn", bufs=12) as wpool, \
             tc.tile_pool(name="an", bufs=12) as apool, \
             tc.tile_pool(name="wb", bufs=12) as wbpool, \
             tc.tile_pool(name="trig", bufs=3) as tpool, \
             tc.tile_pool(name="uv", bufs=4) as uvpool, \
             tc.tile_pool(name="fin", bufs=1) as fpool, \
             tc.tile_pool(name="psy", bufs=2, space="PSUM") as psy, \
             tc.tile_pool(name="psw", bufs=2, space="PSUM") as psw, \
             tc.tile_pool(name="pso", bufs=1, space="PSUM") as pso, \
             tc.tile_pool(name="psf", bufs=1, space="PSUM") as psf:

            ident = cpool.tile([P, P], f32)
            make_identity(nc, ident[:])
            identb = cpool.tile([P, P], bf16)
            make_identity(nc, identb[:])
            ident8 = cpool.tile([8, 8], f32)
            make_identity(nc, ident8[:])

            # ---- stationary trig from full theta (j side) ----
            th_sb = cpool.tile([P, JT * D], f32)       # [p, (t d)]
            nc.sync.dma_start(
                out=th_sb[:].rearrange("p (t d) -> p t d", d=D),
                in_=th_d.ap().rearrange("(t p) d -> p t d", p=P),
            )
            s_th, c_th = sincos(cpool, th_sb[:], JT * D, "th")
            # trigU = [S | C] * (1/N), trigV = [-C | S] * (1/N), per j-tile
            cscale = GLOBAL_COUPLING * STEP_SIZE / float(N)
            trigU = cpool.tile([P, JT * 8], bf16)
            trigV = cpool.tile([P, JT * 8], bf16)
            tU = trigU[:].rearrange("p (t e) -> p t e", e=8)
            tV = trigV[:].rearrange("p (t e) -> p t e", e=8)
            sth3 = s_th[:].rearrange("p (t d) -> p t d", d=D)
            cth3 = c_th[:].rearrange("p (t d) -> p t d", d=D)
            nc.vector.tensor_scalar(tU[:, :, 0:4], sth3, cscale, None, Alu.mult)
            nc.vector.tensor_scalar(tU[:, :, 4:8], cth3, cscale, None, Alu.mult)
            nc.vector.tensor_scalar(tV[:, :, 0:4], cth3, -cscale, None, Alu.mult)
            nc.vector.tensor_scalar(tV[:, :, 4:8], sth3, cscale, None, Alu.mult)

            # ---- own-rows theta/gamma (i side), natural layout ----
            thi = cpool.tile([P, NB * D], f32)
            nc.sync.dma_start(
                out=thi[:].rearrange("p (t d) -> p t d", d=D),
                in_=thi_d.ap().rearrange("(t p) d -> p t d", p=P),
            )
            gmi = cpool.tile([P, NB * D], f32)
            nc.sync.dma_start(
                out=gmi[:].rearrange("p (t d) -> p t d", d=D),
                in_=gm_d.ap().rearrange("(t p) d -> p t d", p=P),
            )
            s_i, c_i = sincos(cpool, thi[:], NB * D, "ti")

            ab_slabs = []
            for s in range(NSLAB):
                psum_out = pso.tile([8, SLAB], f32)
                for k in range(NCHUNK):
                    wb = []
                    an = []
                    for ib in range(4):
                        r0 = s * SLAB + ib * P
                        wt = wpool.tile([P, JC], f32, tag="wn")
                        at = apool.tile([P, JC], f32, tag="an")
                        for h in range(2):
                            c0 = k * JC + h * (JC // 2)
                            nc.sync.dma_start(
                                out=wt[:, h * (JC // 2):(h + 1) * (JC // 2)],
                                in_=w_d.ap()[r0:r0 + P, c0:c0 + JC // 2])
                            nc.sync.dma_start(
                                out=at[:, h * (JC // 2):(h + 1) * (JC // 2)],
                                in_=a_d.ap()[r0:r0 + P, c0:c0 + JC // 2])
                        wbt = wbpool.tile([P, JC], bf16, tag="wb")
                        nc.scalar.copy(out=wbt[:], in_=wt[:])
                        wb.append(wbt)
                        an.append(at)
                    # 8 j-tiles per chunk; process in 4 groups of 2 j-tiles
                    for jg in range(4):
                        psumA = psy.tile([P, 1024], f32)
                        for jt2 in range(2):
                            jl = jg * 2 + jt2
                            for ib in range(4):
                                nc.tensor.transpose(
                                    out=psumA[:, jt2 * 512 + ib * P:
                                              jt2 * 512 + (ib + 1) * P],
                                    in_=an[ib][:, jl * P:(jl + 1) * P],
                                    identity=ident[:],
                                )
                        q2 = tpool.tile([P, 1024], bf16, tag="q2")
                        q4 = tpool.tile([P, 1024], bf16, tag="q4")
                        nc.scalar.activation(q2[:], psumA[:], Act.Sin, scale=0.5)
                        nc.scalar.activation(q4[:], psumA[:], Act.Sin, scale=0.25)
                        r2 = tpool.tile([P, 1024], bf16, tag="r2")
                        cc = tpool.tile([P, 1024], bf16, tag="cc")
                        nc.vector.tensor_tensor(out=r2[:], in0=q2[:], in1=q2[:],
                                                op=Alu.mult)
                        nc.vector.tensor_scalar(cc[:], r2[:], -2.0, 1.0,
                                                Alu.mult, Alu.add)
                        r4 = tpool.tile([P, 1024], bf16, tag="r4")
                        ch = tpool.tile([P, 1024], bf16, tag="ch")
                        nc.vector.tensor_tensor(out=r4[:], in0=q4[:], in1=q4[:],
                                                op=Alu.mult)
                        nc.vector.tensor_scalar(ch[:], r4[:], -4.0, 2.0,
                                                Alu.mult, Alu.add)
                        ss = tpool.tile([P, 1024], bf16, tag="ss")
                        nc.vector.tensor_tensor(out=ss[:], in0=q2[:], in1=ch[:],
                                                op=Alu.mult)
                        psumW = psw.tile([P, 1024], bf16)
                        for jt2 in range(2):
                            jl = jg * 2 + jt2
                            for ib in range(4):
                                nc.tensor.transpose(
                                    out=psumW[:, jt2 * 512 + ib * P:
                                              jt2 * 512 + (ib + 1) * P],
                                    in_=wb[ib][:, jl * P:(jl + 1) * P],
                                    identity=identb[:],
                                )
                        ut = uvpool.tile([P, 1024], bf16, tag="ut")
                        vt = uvpool.tile([P, 1024], bf16, tag="vt")
                        nc.vector.tensor_tensor(out=ut[:], in0=psumW[:],
                                                in1=cc[:], op=Alu.mult)
                        nc.vector.tensor_tensor(out=vt[:], in0=psumW[:],
                                                in1=ss[:], op=Alu.mult)
                        for jt2 in range(2):
                            jt_glob = k * 8 + jg * 2 + jt2
                            first = (k == 0 and jg == 0 and jt2 == 0)
                            last = (k == NCHUNK - 1 and jg == 3 and jt2 == 1)
                            sl = slice(jt2 * 512, (jt2 + 1) * 512)
                            nc.tensor.matmul(
                                out=psum_out[:],
                                lhsT=trigU[:, jt_glob * 8:(jt_glob + 1) * 8],
                                rhs=ut[:, sl],
                                start=first, stop=False,
                            )
                            nc.tensor.matmul(
                                out=psum_out[:],
                                lhsT=trigV[:, jt_glob * 8:(jt_glob + 1) * 8],
                                rhs=vt[:, sl],
                                start=False, stop=last,
                            )
                ob = fpool.tile([8, SLAB], f32, tag=f"ob{s}")
                nc.vector.tensor_copy(out=ob[:], in_=psum_out[:])
                ab_slabs.append(ob)

            # ---- finish: transpose [8,512] -> [128,8], combine, normalize ----
            for s in range(NSLAB):
                ob = ab_slabs[s]
                for ib in range(4):
                    blk = s * 4 + ib
                    psumF = psf.tile([P, 8], f32)
                    nc.tensor.transpose(
                        out=psumF[:],
                        in_=ob[:, ib * P:(ib + 1) * P],
                        identity=ident8[:],
                    )
                    ab = fpool.tile([P, 8], f32, tag="ab")
                    nc.vector.tensor_copy(out=ab[:], in_=psumF[:])
                    t1 = fpool.tile([P, D], f32, tag="t1")
                    t2 = fpool.tile([P, D], f32, tag="t2")
                    x = fpool.tile([P, D], f32, tag="x")
                    csl = c_i[:, blk * D:(blk + 1) * D]
                    ssl = s_i[:, blk * D:(blk + 1) * D]
                    nc.vector.tensor_tensor(out=t1[:], in0=ab[:, 0:4], in1=csl,
                                            op=Alu.mult)
                    nc.vector.tensor_tensor(out=t2[:], in0=ab[:, 4:8], in1=ssl,
                                            op=Alu.mult)
                    nc.vector.tensor_tensor(out=x[:], in0=t1[:], in1=t2[:],
                                            op=Alu.subtract)
                    nc.vector.tensor_tensor(out=x[:], in0=x[:],
                                            in1=gmi[:, blk * D:(blk + 1) * D],
                                            op=Alu.add)
                    sq = fpool.tile([P, D], f32, tag="sq")
                    nc.vector.tensor_tensor(out=sq[:], in0=x[:], in1=x[:],
                                            op=Alu.mult)
                    n2 = fpool.tile([P, 1], f32, tag="n2")
                    nc.vector.tensor_reduce(out=n2[:], in_=sq[:],
                                            axis=mybir.AxisListType.X,
                                            op=Alu.add)
                    nrm = fpool.tile([P, 1], f32, tag="nrm")
                    nc.scalar.activation(nrm[:], n2[:], Act.Sqrt)
                    mx = fpool.tile([P, 1], f32, tag="mx")
                    nc.vector.tensor_scalar(mx[:], nrm[:], EPS, None, Alu.max)
                    rinv = fpool.tile([P, 1], f32, tag="rinv")
                    nc.vector.reciprocal(rinv[:], mx[:])
                    o = fpool.tile([P, D], f32, tag="o")
                    nc.vector.tensor_scalar(o[:], x[:], rinv[:, 0:1], None,
                                            Alu.mult)
                    nc.sync.dma_start(
                        out=out_d.ap()[blk * P:(blk + 1) * P, :],
                        in_=o[:],
                    )

    nc.compile()
    return nc


def _get_nc():
    if "nc" not in _CACHE:
        _CACHE["nc"] = _build()
    return _CACHE["nc"]


def make_in_maps(theta_prev, gamma_prev, theta_connectivity_weight, alpha_t):
    theta_prev = np.ascontiguousarray(theta_prev, dtype=np.float32)
    gamma_prev = np.ascontiguousarray(gamma_prev, dtype=np.float32)
    W = np.ascontiguousarray(theta_connectivity_weight, dtype=np.float32)
    A = np.ascontiguousarray(alpha_t, dtype=np.float32)
    in_maps = []
    for c in range(N_CORES):
        b = c // CORES_PER_BATCH
        r0 = (c % CORES_PER_BATCH) * ROWS
        in_maps.append({
            "w": np.ascontiguousarray(W[b, r0:r0 + ROWS]),
            "alpha": np.ascontiguousarray(A[b, r0:r0 + ROWS]),
            "theta": np.ascontiguousarray(theta_prev[b]),
            "theta_i": np.ascontiguousarray(theta_prev[b, r0:r0 + ROWS]),
            "gamma": np.ascontiguousarray(gamma_prev[b, r0:r0 + ROWS]),
        })
    return in_maps


def kernel(theta_prev, gamma_prev, theta_connectivity_weight, alpha_t):
    from concourse.bass_utils import run_bass_kernel_spmd

    nc = _get_nc()
    in_maps = make_in_maps(theta_prev, gamma_prev,
                           theta_connectivity_weight, alpha_t)
    res = run_bass_kernel_spmd(nc, in_maps, core_ids=list(range(N_CORES)))
    out = np.empty((B, N, D), dtype=np.float32)
    for c in range(N_CORES):
        b = c // CORES_PER_BATCH
        r0 = (c % CORES_PER_BATCH) * ROWS
        out[b, r0:r0 + ROWS] = res.results[c]["out"]
    return out



# revision 11
# speedup vs baseline: 26.4378x; 26.4378x over previous
"""Kuramoto layer Bass/Tile kernel for 8 Trainium2 NeuronCores.

Math: coupling[b,i,d] = (1/N) * sum_j W[b,i,j] * sin(theta[b,j,d] - theta[b,i,d] - alpha[b,i,j])
Using sin(tj - ti - a) = cos(ti)*(sin(tj)cos(a) - cos(tj)sin(a)) - sin(ti)*(cos(tj)cos(a) + sin(tj)sin(a)):
  A[i,d] = sum_j U[i,j] S[j,d] - V[i,j] C[j,d]     U = W cos a, V = W sin a
  B[i,d] = sum_j U[i,j] C[j,d] + V[i,j] S[j,d]
  coupling = cos(ti) * A - sin(ti) * B
  out = normalize(gamma + coupling/N, dim=-1, eps=1e-6)

Host staging (layout/dtype/angle-canonicalization only): W and alpha are
sliced per core, TRANSPOSED to [N, ROWS] (j-major), alpha is reduced to its
canonical angle aw = ((alpha+pi) mod 2pi) - pi in [-pi, pi] (exact mod-2pi
identity), and both are rounded to bf16 / fp16. The device then needs no PE
transposes and no range reduction: j is already the partition dim and every
Sin argument is inside the ACT table's [-pi, pi] domain:
  sin a = Sin(aw)                     [ACT]
  p     = Sin(0.5 * aw)               [ACT]    cos a = 1 - 2 p^2
  V = W*sin a ; Wn = W*p^2            [DVE]
  A/B accumulate via three matmul passes: W @ [S|C], Wn @ -2[S|C], V @ [-C|S]

Sharding: core c handles batch c//4, i-rows (c%4)*1024 .. +1024. theta (j-side)
is replicated per batch. No cross-core communication.
"""

import sys

if "/opt/trn_rl_repo" not in sys.path:
    sys.path.insert(0, "/opt/trn_rl_repo")

import math

import numpy as np
import ml_dtypes

B, N, D = 2, 4096, 4
N_CORES = 8
CORES_PER_BATCH = N_CORES // B          # 4
ROWS = B * N // N_CORES                 # 1024 i-rows per core
P = 128
SLAB = 512                              # i-slab (matmul moving width)
NSLAB = ROWS // SLAB                    # 2
NB = ROWS // P                          # 8 row-blocks per core
JT = N // P                             # 32 j-tiles
PAIR = 2                                # j-tiles per SBUF tile
NPAIR = JT // PAIR                      # 16
FW = PAIR * ROWS                        # 2048 free width per tile
PI = math.pi
EPS = 1e-6
GLOBAL_COUPLING = 1.0
STEP_SIZE = 1.0
GAMMA_STRENGTH = 1.0

_CACHE = {}


def _build():
    from concourse import bacc, mybir
    import concourse.tile as tile
    from concourse.masks import make_identity

    f32 = mybir.dt.float32
    f16 = mybir.dt.float16
    bf16 = mybir.dt.bfloat16
    Alu = mybir.AluOpType
    Act = mybir.ActivationFunctionType

    nc = bacc.Bacc("TRN2", target_bir_lowering=False, debug=False,
                   num_devices=N_CORES)

    w_d = nc.dram_tensor("wT", [N, ROWS], bf16, kind="ExternalInput")
    a_d = nc.dram_tensor("alphaT", [N, ROWS], f16, kind="ExternalInput")
    th_d = nc.dram_tensor("theta", [N, D], f32, kind="ExternalInput")
    thi_d = nc.dram_tensor("theta_i", [ROWS, D], f32, kind="ExternalInput")
    gm_d = nc.dram_tensor("gamma", [ROWS, D], f32, kind="ExternalInput")
    out_d = nc.dram_tensor("out", [ROWS, D], f32, kind="ExternalOutput")

    def sincos(pool, src, width, tag):
        """f32 sin/cos of src [P, width] via half-angle; returns (sin, cos)."""
        q2 = pool.tile([P, width], f32, tag=f"{tag}q2")
        q4 = pool.tile([P, width], f32, tag=f"{tag}q4")
        nc.scalar.activation(q2[:], src, Act.Sin, scale=0.5)
        nc.scalar.activation(q4[:], src, Act.Sin, scale=0.25)
        cos_t = pool.tile([P, width], f32, tag=f"{tag}cos")
        r2 = pool.tile([P, width], f32, tag=f"{tag}r2")
        nc.vector.tensor_tensor(out=r2[:], in0=q2[:], in1=q2[:], op=Alu.mult)
        nc.vector.tensor_scalar(cos_t[:], r2[:], -2.0, 1.0, Alu.mult, Alu.add)
        r4 = pool.tile([P, width], f32, tag=f"{tag}r4")
        ch = pool.tile([P, width], f32, tag=f"{tag}ch")
        nc.vector.tensor_tensor(out=r4[:], in0=q4[:], in1=q4[:], op=Alu.mult)
        nc.vector.tensor_scalar(ch[:], r4[:], -4.0, 2.0, Alu.mult, Alu.add)
        sin_t = pool.tile([P, width], f32, tag=f"{tag}sin")
        nc.vector.tensor_tensor(out=sin_t[:], in0=q2[:], in1=ch[:], op=Alu.mult)
        return sin_t, cos_t

    with tile.TileContext(nc) as tc:
        with tc.tile_pool(name="const", bufs=1) as cpool, \
             tc.tile_pool(name="wn", bufs=4) as wpool, \
             tc.tile_pool(name="an", bufs=4) as apool, \
             tc.tile_pool(name="trig", bufs=3) as tpool, \
             tc.tile_pool(name="uv", bufs=3) as uvpool, \
             tc.tile_pool(name="fin", bufs=1) as fpool, \
             tc.tile_pool(name="pso", bufs=1, space="PSUM") as pso, \
             tc.tile_pool(name="psf", bufs=1, space="PSUM") as psf:

            ident8 = cpool.tile([8, 8], f32)
            make_identity(nc, ident8[:])

            # ---- stationary trig from full theta (j side) ----
            th_sb = cpool.tile([P, JT * D], f32)       # [p, (t d)]
            nc.sync.dma_start(
                out=th_sb[:].rearrange("p (t d) -> p t d", d=D),
                in_=th_d.ap().rearrange("(t p) d -> p t d", p=P),
            )
            s_th, c_th = sincos(cpool, th_sb[:], JT * D, "th")
            # trigU = [S | C] * (1/N), trigV = [-C | S] * (1/N), per j-tile
            cscale = GLOBAL_COUPLING * STEP_SIZE / float(N)
            trigU = cpool.tile([P, JT * 8], bf16)
            trigM = cpool.tile([P, JT * 8], bf16)
            trigV = cpool.tile([P, JT * 8], bf16)
            tU = trigU[:].rearrange("p (t e) -> p t e", e=8)
            tM = trigM[:].rearrange("p (t e) -> p t e", e=8)
            tV = trigV[:].rearrange("p (t e) -> p t e", e=8)
            sth3 = s_th[:].rearrange("p (t d) -> p t d", d=D)
            cth3 = c_th[:].rearrange("p (t d) -> p t d", d=D)
            nc.vector.tensor_scalar(tU[:, :, 0:4], sth3, cscale, None, Alu.mult)
            nc.vector.tensor_scalar(tU[:, :, 4:8], cth3, cscale, None, Alu.mult)
            nc.vector.tensor_scalar(tM[:, :, 0:4], sth3, -2.0 * cscale, None,
                                    Alu.mult)
            nc.vector.tensor_scalar(tM[:, :, 4:8], cth3, -2.0 * cscale, None,
                                    Alu.mult)
            nc.vector.tensor_scalar(tV[:, :, 0:4], cth3, -cscale, None, Alu.mult)
            nc.vector.tensor_scalar(tV[:, :, 4:8], sth3, cscale, None, Alu.mult)

            # ---- own-rows theta/gamma (i side), natural layout ----
            thi = cpool.tile([P, NB * D], f32)
            nc.sync.dma_start(
                out=thi[:].rearrange("p (t d) -> p t d", d=D),
                in_=thi_d.ap().rearrange("(t p) d -> p t d", p=P),
            )
            gmi = cpool.tile([P, NB * D], f32)
            nc.sync.dma_start(
                out=gmi[:].rearrange("p (t d) -> p t d", d=D),
                in_=gm_d.ap().rearrange("(t p) d -> p t d", p=P),
            )
            s_i, c_i = sincos(cpool, thi[:], NB * D, "ti")

            psum0 = pso.tile([8, SLAB], f32, tag="o0")
            psum1 = pso.tile([8, SLAB], f32, tag="o1")
            psums = [psum0, psum1]

            for pr in range(NPAIR):
                j0 = pr * PAIR * P
                wt = wpool.tile([P, FW], bf16, tag="wn")
                at = apool.tile([P, FW], f16, tag="an")
                nc.sync.dma_start(
                    out=wt[:].rearrange("p (t i) -> p t i", t=PAIR),
                    in_=w_d.ap()[j0:j0 + PAIR * P, :]
                        .rearrange("(t p) i -> p t i", p=P),
                )
                nc.sync.dma_start(
                    out=at[:].rearrange("p (t i) -> p t i", t=PAIR),
                    in_=a_d.ap()[j0:j0 + PAIR * P, :]
                        .rearrange("(t p) i -> p t i", p=P),
                )
                sa = tpool.tile([P, FW], bf16, tag="sa")
                p_t = tpool.tile([P, FW], bf16, tag="p")
                nc.scalar.activation(sa[:], at[:], Act.Sin)
                nc.scalar.activation(p_t[:], at[:], Act.Sin, scale=0.5)
                n_t = uvpool.tile([P, FW], bf16, tag="n")
                vt = uvpool.tile([P, FW], bf16, tag="vt")
                un = uvpool.tile([P, FW], bf16, tag="un")
                nc.vector.tensor_tensor(out=n_t[:], in0=p_t[:], in1=p_t[:],
                                        op=Alu.mult)
                nc.vector.tensor_tensor(out=vt[:], in0=wt[:], in1=sa[:],
                                        op=Alu.mult)
                nc.vector.tensor_tensor(out=un[:], in0=wt[:], in1=n_t[:],
                                        op=Alu.mult)
                for t in range(PAIR):
                    jt = pr * PAIR + t
                    for s in range(NSLAB):
                        first = (pr == 0 and t == 0)
                        last = (pr == NPAIR - 1 and t == PAIR - 1)
                        sl = slice(t * ROWS + s * SLAB,
                                   t * ROWS + (s + 1) * SLAB)
                        nc.tensor.matmul(
                            out=psums[s][:],
                            lhsT=trigU[:, jt * 8:(jt + 1) * 8],
                            rhs=wt[:, sl],
                            start=first, stop=False,
                        )
                        nc.tensor.matmul(
                            out=psums[s][:],
                            lhsT=trigM[:, jt * 8:(jt + 1) * 8],
                            rhs=un[:, sl],
                            start=False, stop=False,
                        )
                        nc.tensor.matmul(
                            out=psums[s][:],
                            lhsT=trigV[:, jt * 8:(jt + 1) * 8],
                            rhs=vt[:, sl],
                            start=False, stop=last,
                        )

            # ---- finish: transpose [8,512] -> [128, 8 per blk], batched ----
            ab_slabs = []
            for s in range(NSLAB):
                ob = fpool.tile([8, SLAB], f32, tag=f"ob{s}")
                nc.vector.tensor_copy(out=ob[:], in_=psums[s][:])
                ab_slabs.append(ob)
            psumF = psf.tile([P, NB * 8], f32)
            for s in range(NSLAB):
                ob = ab_slabs[s]
                for ib in range(4):
                    blk = s * 4 + ib
                    nc.tensor.transpose(
                        out=psumF[:, blk * 8:(blk + 1) * 8],
                        in_=ob[:, ib * P:(ib + 1) * P],
                        identity=ident8[:],
                    )
            ab = fpool.tile([P, NB * 8], f32, tag="ab")
            nc.vector.tensor_copy(out=ab[:], in_=psumF[:])
            ab3 = ab[:].rearrange("p (t e) -> p t e", e=8)
            ci3 = c_i[:].rearrange("p (t d) -> p t d", d=D)
            si3 = s_i[:].rearrange("p (t d) -> p t d", d=D)
            t1 = fpool.tile([P, NB * D], f32, tag="t1")
            t2 = fpool.tile([P, NB * D], f32, tag="t2")
            x = fpool.tile([P, NB * D], f32, tag="x")
            t13 = t1[:].rearrange("p (t d) -> p t d", d=D)
            t23 = t2[:].rearrange("p (t d) -> p t d", d=D)
            nc.vector.tensor_tensor(out=t13, in0=ab3[:, :, 0:4], in1=ci3,
                                    op=Alu.mult)
            nc.vector.tensor_tensor(out=t23, in0=ab3[:, :, 4:8], in1=si3,
                                    op=Alu.mult)
            nc.vector.tensor_tensor(out=x[:], in0=t1[:], in1=t2[:],
                                    op=Alu.subtract)
            nc.vector.tensor_tensor(out=x[:], in0=x[:], in1=gmi[:],
                                    op=Alu.add)
            sq = fpool.tile([P, NB * D], f32, tag="sq")
            nc.vector.tensor_tensor(out=sq[:], in0=x[:], in1=x[:], op=Alu.mult)
            sq3 = sq[:].rearrange("p (t d) -> p t d", d=D)
            n2 = fpool.tile([P, NB], f32, tag="n2")
            n23 = n2[:].rearrange("p (t e) -> p t e", e=1)
            nc.vector.tensor_reduce(out=n23, in_=sq3,
                                    axis=mybir.AxisListType.X, op=Alu.add)
            nrm = fpool.tile([P, NB], f32, tag="nrm")
            nc.scalar.activation(nrm[:], n2[:], Act.Sqrt)
            mx = fpool.tile([P, NB], f32, tag="mx")
            nc.vector.tensor_scalar(mx[:], nrm[:], EPS, None, Alu.max)
            rinv = fpool.tile([P, NB], f32, tag="rinv")
            nc.vector.reciprocal(rinv[:], mx[:])
            o = fpool.tile([P, NB * D], f32, tag="o")
            for blk in range(NB):
                nc.vector.tensor_scalar(o[:, blk * D:(blk + 1) * D],
                                        x[:, blk * D:(blk + 1) * D],
                                        rinv[:, blk:blk + 1], None, Alu.mult)
            nc.sync.dma_start(
                out=out_d.ap().rearrange("(t p) d -> p t d", p=P),
                in_=o[:].rearrange("p (t d) -> p t d", d=D),
            )

    nc.compile()
    return nc


def _get_nc():
    if "nc" not in _CACHE:
        _CACHE["nc"] = _build()
    return _CACHE["nc"]


def make_in_maps(theta_prev, gamma_prev, theta_connectivity_weight, alpha_t):
    theta_prev = np.ascontiguousarray(theta_prev, dtype=np.float32)
    gamma_prev = np.ascontiguousarray(gamma_prev, dtype=np.float32)
    W = np.asarray(theta_connectivity_weight, dtype=np.float32)
    A = np.asarray(alpha_t, dtype=np.float32)
    in_maps = []
    for c in range(N_CORES):
        b = c // CORES_PER_BATCH
        r0 = (c % CORES_PER_BATCH) * ROWS
        in_maps.append({
            "wT": W[b, r0:r0 + ROWS].T.astype(ml_dtypes.bfloat16),
            "alphaT": (np.mod(A[b, r0:r0 + ROWS].T + np.float32(PI),
                              np.float32(2 * PI)) - np.float32(PI)
                       ).astype(np.float16),
            "theta": np.ascontiguousarray(theta_prev[b]),
            "theta_i": np.ascontiguousarray(theta_prev[b, r0:r0 + ROWS]),
            "gamma": np.ascontiguousarray(gamma_prev[b, r0:r0 + ROWS]),
        })
    return in_maps


def kernel(theta_prev, gamma_prev, theta_connectivity_weight, alpha_t):
    from concourse.bass_utils import run_bass_kernel_spmd

    nc = _get_nc()
    in_maps = make_in_maps(theta_prev, gamma_prev,
                           theta_connectivity_weight, alpha_t)
    res = run_bass_kernel_spmd(nc, in_maps, core_ids=list(range(N_CORES)))
    out = np.empty((B, N, D), dtype=np.float32)
    for c in range(N_CORES):
        b = c // CORES_PER_BATCH
        r0 = (c % CORES_PER_BATCH) * ROWS
        out[b, r0:r0 + ROWS] = res.results[c]["out"]
    return out
